# revision 18
# baseline (speedup 1.0000x reference)
"""Trainium2 Bass kernel for a transformer attention block (BasicBlock).

Reference computation (B=2, L=2048, D=1024, H=16, C=64):
    qkv = x @ w_qkv.T + b_qkv ; q,k,v = split(qkv)
    attn = softmax((q @ k.T) / sqrt(D)) ; heads = attn @ v
    out  = heads @ w_o.T + b_o + x

Sharding: 8 cores = 2 batches x 4 head-groups (4 heads each).
Per core (b, g):
    qkvT = w_qkv_g @ x_b.T (+bias for q,k at eviction)    [768, 2048]
    V    = transpose(V^T) via PE                          [2048, 4x65]
    S^T_h = zero-padded K=128 matmuls (K^T_h stationary)  per (h, l-chunk)
    P^T  = exp(S^T * scale)   (no max-subtraction; scores bounded ~±1)
    O'^T = [V_h | 1]^T @ P^T  -> rows 0..63 = O^T, row 64 = denominators
    normalize via reciprocal + partition_broadcast, place in ot via SBUF DMA
    partial = O @ w_o[:, cols_g].T                        [2048, 1024]
Host: sum 4 group partials per batch, add x + b_o + w_o @ b_v.
"""

import sys

if "/opt/trn_rl_repo" not in sys.path:
    sys.path.insert(0, "/opt/trn_rl_repo")

import numpy as np

B, L, D, H = 2, 2048, 1024, 16
C = 64
HPC = 4            # heads per core
G = 256            # dims per head group (HPC * C)
SCALE = float(1.0 / np.sqrt(np.float32(D)))

LC = 512           # l-chunk (moving dim)
NLC = L // LC      # 4
MT = L // 128      # 16 m-tiles
DT = D // 128      # 8 d-tiles
NEC = D // 512     # 2 e-chunks for out projection

_CACHE = {}

ALL_PHASES = ("p1", "vt", "scores", "exp", "av", "p4")


def _build(reps=1, phases=ALL_PHASES):
    import concourse.mybir as mybir
    import concourse.tile as tile
    from concourse import bacc
    from concourse.masks import make_identity
    from contextlib import ExitStack

    f32 = mybir.dt.float32
    f32r = mybir.dt.float32r
    Exp = mybir.ActivationFunctionType.Exp

    nc = bacc.Bacc("TRN2", target_bir_lowering=False, debug=False)

    xT = nc.declare_dram_parameter("xT", [D, L], f32r, isOutput=False)
    # columns: [Q (256) | K (256) | V (256)] of this head group, transposed
    wqkvT = nc.declare_dram_parameter("wqkvT", [D, 3 * G], f32r, isOutput=False)
    bqk = nc.declare_dram_parameter("bqk", [128, 4], f32, isOutput=False)
    woT = nc.declare_dram_parameter("woT", [G, D], f32r, isOutput=False)
    out = nc.declare_dram_parameter("out", [L, D], f32, isOutput=True)

    with tile.TileContext(nc) as tc:
      for _rep in range(reps):
        with (
            tc.tile_pool(name="const", bufs=1) as constp,
            tc.tile_pool(name="qp", bufs=2) as qpp,
            tc.tile_pool(name="kz", bufs=4) as kzp,
            tc.tile_pool(name="vt", bufs=16) as vtp,
            tc.tile_pool(name="wo", bufs=2) as wop,
            tc.tile_pool(name="ot", bufs=2) as otp,
            tc.tile_pool(name="ps_mm", bufs=2, space="PSUM") as psmm,
            tc.tile_pool(name="ps_sc", bufs=2, space="PSUM") as pssc,
            tc.tile_pool(name="ps_o", bufs=2, space="PSUM") as pso,
        ):
            bqk_sb = constp.tile([128, 4], f32)
            nc.sync.dma_start(out=bqk_sb[:], in_=bqk[:])
            ident_f32 = constp.tile([128, 128], f32, name="ident_f32")
            make_identity(nc, ident_f32)
            ident = constp.tile([128, 128], f32r, name="ident")
            nc.vector.tensor_copy(ident[:], ident_f32[:])

            wo_sb = []
            for t in range(2):
                w = wop.tile([128, D], f32r, name="wo_sb", tag="wo_sb")
                nc.sync.dma_start(out=w[:], in_=woT[t * 128:(t + 1) * 128, :])
                wo_sb.append(w)

            # qp[p]: Q^T pair tiles (partitions 0-63 head 2p, 64-127 head 2p+1)
            qp = [qpp.tile([128, L], f32r, name="qp", tag="qp") for _ in range(2)]
            # kz[h]: K^T_h zero-padded to 128 partitions at its parity offset
            kz = [kzp.tile([128, L], f32r, name="kz", tag="kz") for _ in range(HPC)]
            # v[mt]: [128, 4*65]; per head block: [V_h (64 cols) | ones]
            vt = [vtp.tile([128, HPC * 65], f32r, name="vt", tag="vt") for _ in range(MT)]
            ot = [otp.tile([128, L], f32r, name="ot", tag="ot") for _ in range(2)]

            with (
                tc.tile_pool(name="xt", bufs=DT) as xtp,
                tc.tile_pool(name="wqkv", bufs=DT) as wqkvp,
                tc.tile_pool(name="vsT", bufs=2) as vstp,
            ):
                xt, wq = [], []
                for i in range(DT):
                    x_sb = xtp.tile([128, L], f32r, name="x_sb", tag="x_sb")
                    for c in range(NLC):
                        cs = slice(c * LC, (c + 1) * LC)
                        nc.sync.dma_start(out=x_sb[:, cs], in_=xT[i * 128:(i + 1) * 128, cs])
                    xt.append(x_sb)
                    w = wqkvp.tile([128, 3 * G], f32r, name="wqkv_sb", tag="wqkv_sb")
                    nc.sync.dma_start(out=w[:], in_=wqkvT[i * 128:(i + 1) * 128, :])
                    wq.append(w)

                # zero the unused parity halves of kz (memset cannot write
                # f32r; multiply loaded data by 0 instead)
                for h in range(HPC):
                    zs = slice(64, 128) if h % 2 == 0 else slice(0, 64)
                    nc.gpsimd.tensor_scalar_mul(kz[h][zs, :], xt[0][zs, :], 0.0)

                vsT = [vstp.tile([128, L], f32r, name="vsT", tag="vsT") for _ in range(2)]

                # ---- P1: qkvT = wqkv^T.T @ xT ----
                # t: 0,1 = Q pairs; 2,3 = K pairs; 4,5 = V^T tiles.
                # K first so attention units for heads 0/1 can start early.
                for t in ([2, 0, 4, 5, 3, 1] if "p1" in phases else []):
                    for lc in range(NLC):
                        ps = psmm.tile([128, LC], f32, name="ps", tag="ps")
                        for d in range(DT):
                            nc.tensor.matmul(
                                ps[:],
                                lhsT=wq[d][:, t * 128:(t + 1) * 128],
                                rhs=xt[d][:, lc * LC:(lc + 1) * LC],
                                start=(d == 0),
                                stop=(d == DT - 1),
                            )
                        ls = slice(lc * LC, (lc + 1) * LC)
                        if t < 2:
                            nc.vector.tensor_scalar_add(
                                qp[t][:, ls], ps[:], bqk_sb[:, t:t + 1]
                            )
                        elif t < 4:
                            h0 = 2 * (t - 2)
                            nc.vector.tensor_scalar_add(
                                kz[h0][0:64, ls], ps[0:64, :], bqk_sb[0:64, t:t + 1]
                            )
                            nc.vector.tensor_scalar_add(
                                kz[h0 + 1][64:128, ls], ps[64:128, :],
                                bqk_sb[64:128, t:t + 1],
                            )
                        else:
                            nc.vector.tensor_copy(vsT[t - 4][:, ls], ps[:])

                # ---- V = transpose(V^T) per m-tile, head-blocked with ones ----
                for mt in (range(MT) if "vt" in phases else []):
                    ms = slice(mt * 128, (mt + 1) * 128)
                    v3d = vt[mt][:].rearrange("p (h c) -> p h c", h=HPC)
                    for et in range(2):
                        pst = psmm.tile([128, 128], f32r, name="pst", tag="ps")
                        nc.tensor.transpose(pst[:], vsT[et][:, ms], ident[:])
                        nc.vector.tensor_copy(
                            v3d[:, 2 * et:2 * et + 2, 0:64],
                            pst[:].rearrange("p (h c) -> p h c", h=2),
                        )
                    nc.vector.tensor_scalar(
                        v3d[:, :, 64:65], v3d[:, :, 0:1], 0.0, 1.0,
                        mybir.AluOpType.mult, mybir.AluOpType.add,
                    )

            _p34 = ExitStack()
            ptp = _p34.enter_context(tc.tile_pool(name="pt", bufs=12))
            rcpp = _p34.enter_context(tc.tile_pool(name="rcp", bufs=3))
            nrmp = _p34.enter_context(tc.tile_pool(name="nrm", bufs=3))
            stgp = _p34.enter_context(tc.tile_pool(name="stg", bufs=4))

            # ---- P3: attention per (head, l-chunk) ----
            pts = None
            for h in (range(HPC) if "scores" in phases else []):
                po_off = (h % 2) * 64
                for lc in range(NLC):
                    ls = slice(lc * LC, (lc + 1) * LC)
                    pts = []
                    for j in range(MT // 2):
                        ps = pssc.tile([128, 2 * LC], f32)
                        for half in range(2):
                            mt = 2 * j + half
                            nc.tensor.matmul(
                                ps[:, half * LC:(half + 1) * LC],
                                lhsT=kz[h][:, mt * 128:(mt + 1) * 128],
                                rhs=qp[h // 2][:, ls],
                                start=True,
                                stop=True,
                            )
                        ptile = ptp.tile([128, 2 * LC], f32r)
                        if "exp" in phases:
                            nc.scalar.activation(ptile[:], ps[:], Exp, scale=SCALE)
                        else:
                            nc.vector.tensor_copy(ptile[:, 0:8], ps[:, 0:8])
                        pts.append(ptile)

                    if "av" not in phases:
                        continue
                    po = pso.tile([65, LC], f32)
                    for j in range(MT // 2):
                        for half in range(2):
                            mt = 2 * j + half
                            nc.tensor.matmul(
                                po[:],
                                lhsT=vt[mt][:, h * 65:(h + 1) * 65],
                                rhs=pts[j][:, half * LC:(half + 1) * LC],
                                start=(mt == 0),
                                stop=(mt == MT - 1),
                            )

                    # normalize: rows 0..63 = O^T, row 64 = denominators
                    rc = rcpp.tile([128, LC], f32)
                    nc.vector.reciprocal(rc[64:65, :], po[64:65, :])
                    # partition_broadcast reads physical partition 0 on HW;
                    # stage the reciprocal row there via a small SBUF DMA
                    rc0 = rcpp.tile([1, LC], f32, name="rc0")
                    nc.sync.dma_start(out=rc0[0:1, :], in_=rc[64:65, :])
                    rb = rcpp.tile([64, LC], f32)
                    nc.gpsimd.partition_broadcast(rb[:], rc0[0:1, :])
                    nt = nrmp.tile([64, LC], f32r)
                    nc.vector.tensor_mul(nt[:], po[0:64, :], rb[:])
                    nc.sync.dma_start(
                        out=ot[h // 2][po_off:po_off + 64, ls], in_=nt[:]
                    )

            # ---- P4: out = O @ woT (direct PSUM -> DRAM DMA) ----
            if "p4" not in phases:
                if "p1" in phases:
                    nc.sync.dma_start(out=out[128:256, 0:512],
                                      in_=qp[0][:, 0:512].bitcast(f32))
                if "av" in phases:
                    nc.sync.dma_start(out=out[0:128, 0:512],
                                      in_=ot[0][:, 0:512].bitcast(f32))
                elif "scores" in phases and pts:
                    nc.sync.dma_start(out=out[0:128, 0:512],
                                      in_=pts[0][:, 0:512].bitcast(f32))
                if "vt" in phases:
                    nc.sync.dma_start(out=out[256:384, 0:260],
                                      in_=vt[0][:].bitcast(f32))
            for lt in (range(MT) if "p4" in phases else []):
                for ec in range(NEC):
                    ps = psmm.tile([128, 512], f32, name="ps4", tag="ps")
                    for t in range(2):
                        nc.tensor.matmul(
                            ps[:],
                            lhsT=ot[t][:, lt * 128:(lt + 1) * 128],
                            rhs=wo_sb[t][:, ec * 512:(ec + 1) * 512],
                            start=(t == 0),
                            stop=(t == 1),
                        )
                    st = stgp.tile([128, 512], f32)
                    nc.vector.tensor_copy(st[:], ps[:])
                    nc.sync.dma_start(
                        out=out[lt * 128:(lt + 1) * 128, ec * 512:(ec + 1) * 512],
                        in_=st[:],
                    )
            _p34.close()

    nc.compile()
    return nc


def _prep_in_maps(x, w_qkv, b_qkv, w_o):
    xT = [np.ascontiguousarray(x[b].T) for b in range(B)]
    in_maps = []
    for core in range(8):
        b, g = divmod(core, 4)
        qs, ks, vs = g * G, D + g * G, 2 * D + g * G
        wqkvT = np.ascontiguousarray(
            np.concatenate(
                [w_qkv[qs:qs + G], w_qkv[ks:ks + G], w_qkv[vs:vs + G]], axis=0
            ).T
        )
        bqk_m = np.ascontiguousarray(
            np.concatenate([b_qkv[qs:qs + G], b_qkv[ks:ks + G]]).reshape(4, 128).T
        )
        woT = np.ascontiguousarray(w_o[:, g * G:(g + 1) * G].T)
        in_maps.append({"xT": xT[b], "wqkvT": wqkvT, "bqk": bqk_m, "woT": woT})
    return in_maps


def kernel(x, w_qkv, b_qkv, w_o, b_o):
    from concourse.bass_utils import run_bass_kernel_spmd

    x = np.asarray(x, dtype=np.float32)
    w_qkv = np.asarray(w_qkv, dtype=np.float32)
    b_qkv = np.asarray(b_qkv, dtype=np.float32)
    w_o = np.asarray(w_o, dtype=np.float32)
    b_o = np.asarray(b_o, dtype=np.float32)

    if "nc" not in _CACHE:
        _CACHE["nc"] = _build()
    nc = _CACHE["nc"]

    in_maps = _prep_in_maps(x, w_qkv, b_qkv, w_o)
    res = run_bass_kernel_spmd(nc, in_maps, list(range(8)))
    partial = np.stack([res.results[i]["out"] for i in range(8)])  # [8, L, D]

    const = w_o @ b_qkv[2 * D:] + b_o  # [D]
    out = partial.reshape(B, 4, L, D).sum(axis=1) + x + const[None, None, :]
    return out.astype(np.float32)


# revision 19
# speedup vs baseline: 1.0372x; 1.0372x over previous
"""Trainium2 Bass kernel for a transformer attention block (BasicBlock).

Reference computation (B=2, L=2048, D=1024, H=16, C=64):
    qkv = x @ w_qkv.T + b_qkv ; q,k,v = split(qkv)
    attn = softmax((q @ k.T) / sqrt(D)) ; heads = attn @ v
    out  = heads @ w_o.T + b_o + x

Sharding: 8 cores = 2 batches x 4 head-groups (4 heads each).
Per core (b, g):
    qkvT = w_qkv_g @ x_b.T (+bias for q,k at eviction)    [768, 2048]
    V    = transpose(V^T) via PE                          [2048, 4x65]
    S^T_h = zero-padded K=128 matmuls (K^T_h stationary)  per (h, l-chunk)
    P^T  = exp(S^T * scale)   (no max-subtraction; scores bounded ~±1)
    O'^T = [V_h | 1]^T @ P^T  -> rows 0..63 = O^T, row 64 = denominators
    normalize via reciprocal + partition_broadcast, place in ot via SBUF DMA
    partial = O @ w_o[:, cols_g].T                        [2048, 1024]
Host: sum 4 group partials per batch, add x + b_o + w_o @ b_v.
"""

import sys

if "/opt/trn_rl_repo" not in sys.path:
    sys.path.insert(0, "/opt/trn_rl_repo")

import numpy as np

B, L, D, H = 2, 2048, 1024, 16
C = 64
HPC = 4            # heads per core
G = 256            # dims per head group (HPC * C)
SCALE = float(1.0 / np.sqrt(np.float32(D)))

LC = 512           # l-chunk (moving dim)
NLC = L // LC      # 4
MT = L // 128      # 16 m-tiles
DT = D // 128      # 8 d-tiles
NEC = D // 512     # 2 e-chunks for out projection

_CACHE = {}

ALL_PHASES = ("p1", "vt", "scores", "exp", "av", "p4")


def _build(reps=1, phases=ALL_PHASES):
    import concourse.mybir as mybir
    import concourse.tile as tile
    from concourse import bacc
    from concourse.masks import make_identity
    from contextlib import ExitStack

    f32 = mybir.dt.float32
    f32r = mybir.dt.float32r
    Exp = mybir.ActivationFunctionType.Exp

    nc = bacc.Bacc("TRN2", target_bir_lowering=False, debug=False)

    xT = nc.declare_dram_parameter("xT", [D, L], f32r, isOutput=False)
    # columns: [Q (256) | K (256) | V (256)] of this head group, transposed
    wqkvT = nc.declare_dram_parameter("wqkvT", [D, 3 * G], f32r, isOutput=False)
    bqk = nc.declare_dram_parameter("bqk", [128, 4], f32, isOutput=False)
    woT = nc.declare_dram_parameter("woT", [G, D], f32r, isOutput=False)
    out = nc.declare_dram_parameter("out", [L, D], f32, isOutput=True)

    with tile.TileContext(nc) as tc:
      for _rep in range(reps):
        with (
            tc.tile_pool(name="const", bufs=1) as constp,
            tc.tile_pool(name="qp", bufs=2) as qpp,
            tc.tile_pool(name="kz", bufs=4) as kzp,
            tc.tile_pool(name="vt", bufs=16) as vtp,
            tc.tile_pool(name="wo", bufs=2) as wop,
            tc.tile_pool(name="ot", bufs=2) as otp,
            tc.tile_pool(name="ps_mm", bufs=2, space="PSUM") as psmm,
            tc.tile_pool(name="ps_sc", bufs=2, space="PSUM") as pssc,
            tc.tile_pool(name="ps_o", bufs=2, space="PSUM") as pso,
        ):
            bqk_sb = constp.tile([128, 4], f32)
            nc.sync.dma_start(out=bqk_sb[:], in_=bqk[:])

            wo_sb = []
            for t in range(2):
                w = wop.tile([128, D], f32r, name="wo_sb", tag="wo_sb")
                nc.sync.dma_start(out=w[:], in_=woT[t * 128:(t + 1) * 128, :])
                wo_sb.append(w)

            # qp[p]: Q^T pair tiles (partitions 0-63 head 2p, 64-127 head 2p+1)
            qp = [qpp.tile([128, L], f32r, name="qp", tag="qp") for _ in range(2)]
            # kz[h]: K^T_h zero-padded to 128 partitions at its parity offset
            kz = [kzp.tile([128, L], f32r, name="kz", tag="kz") for _ in range(HPC)]
            # v[mt]: [128, 4*65]; per head block: [V_h (64 cols) | ones]
            vt = [vtp.tile([128, HPC * 65], f32r, name="vt", tag="vt") for _ in range(MT)]
            ot = [otp.tile([128, L], f32r, name="ot", tag="ot") for _ in range(2)]

            with (
                tc.tile_pool(name="xt", bufs=DT) as xtp,
                tc.tile_pool(name="wqkv", bufs=DT) as wqkvp,
            ):
                xt, wq = [], []
                for i in range(DT):
                    x_sb = xtp.tile([128, L], f32r, name="x_sb", tag="x_sb")
                    for c in range(NLC):
                        cs = slice(c * LC, (c + 1) * LC)
                        nc.sync.dma_start(out=x_sb[:, cs], in_=xT[i * 128:(i + 1) * 128, cs])
                    xt.append(x_sb)
                    w = wqkvp.tile([128, 3 * G], f32r, name="wqkv_sb", tag="wqkv_sb")
                    nc.sync.dma_start(out=w[:], in_=wqkvT[i * 128:(i + 1) * 128, :])
                    wq.append(w)

                # zero the unused parity halves of kz (memset cannot write
                # f32r; multiply loaded data by 0 instead)
                for h in range(HPC):
                    zs = slice(64, 128) if h % 2 == 0 else slice(0, 64)
                    nc.gpsimd.tensor_scalar_mul(kz[h][zs, :], xt[0][zs, :], 0.0)

                # ---- P1: qkvT = wqkv^T.T @ xT ----
                # t: 0,1 = Q pairs; 2,3 = K pairs; 4,5 = V^T tiles.
                # K first so attention units for heads 0/1 can start early.
                for t in ([2, 0, 3, 1] if "p1" in phases else []):
                    for lc in range(NLC):
                        ps = psmm.tile([128, LC], f32, name="ps", tag="ps")
                        for d in range(DT):
                            nc.tensor.matmul(
                                ps[:],
                                lhsT=wq[d][:, t * 128:(t + 1) * 128],
                                rhs=xt[d][:, lc * LC:(lc + 1) * LC],
                                start=(d == 0),
                                stop=(d == DT - 1),
                            )
                        ls = slice(lc * LC, (lc + 1) * LC)
                        if t < 2:
                            nc.vector.tensor_scalar_add(
                                qp[t][:, ls], ps[:], bqk_sb[:, t:t + 1]
                            )
                        else:
                            h0 = 2 * (t - 2)
                            nc.vector.tensor_scalar_add(
                                kz[h0][0:64, ls], ps[0:64, :], bqk_sb[0:64, t:t + 1]
                            )
                            nc.vector.tensor_scalar_add(
                                kz[h0 + 1][64:128, ls], ps[64:128, :],
                                bqk_sb[64:128, t:t + 1],
                            )

                # ---- P2: V = xT.T @ wv (direct, N=256) ----
                for mt in (range(MT) if "vt" in phases else []):
                    ps = psmm.tile([128, G], f32, name="ps", tag="ps")
                    for d in range(DT):
                        nc.tensor.matmul(
                            ps[:],
                            lhsT=xt[d][:, mt * 128:(mt + 1) * 128],
                            rhs=wq[d][:, 2 * G:3 * G],
                            start=(d == 0),
                            stop=(d == DT - 1),
                        )
                    v3d = vt[mt][:].rearrange("p (h c) -> p h c", h=HPC)
                    nc.vector.tensor_copy(
                        v3d[:, :, 0:64], ps[:].rearrange("p (h c) -> p h c", h=HPC)
                    )
                    nc.vector.tensor_scalar(
                        v3d[:, :, 64:65], v3d[:, :, 0:1], 0.0, 1.0,
                        mybir.AluOpType.mult, mybir.AluOpType.add,
                    )

            _p34 = ExitStack()
            ptp = _p34.enter_context(tc.tile_pool(name="pt", bufs=12))
            rcpp = _p34.enter_context(tc.tile_pool(name="rcp", bufs=3))
            nrmp = _p34.enter_context(tc.tile_pool(name="nrm", bufs=3))
            stgp = _p34.enter_context(tc.tile_pool(name="stg", bufs=4))

            # ---- P3: attention per (head, l-chunk) ----
            pts = None
            for h in (range(HPC) if "scores" in phases else []):
                po_off = (h % 2) * 64
                for lc in range(NLC):
                    ls = slice(lc * LC, (lc + 1) * LC)
                    pts = []
                    for j in range(MT // 2):
                        ps = pssc.tile([128, 2 * LC], f32)
                        for half in range(2):
                            mt = 2 * j + half
                            nc.tensor.matmul(
                                ps[:, half * LC:(half + 1) * LC],
                                lhsT=kz[h][:, mt * 128:(mt + 1) * 128],
                                rhs=qp[h // 2][:, ls],
                                start=True,
                                stop=True,
                            )
                        ptile = ptp.tile([128, 2 * LC], f32r)
                        if "exp" in phases:
                            nc.scalar.activation(ptile[:], ps[:], Exp, scale=SCALE)
                        else:
                            nc.vector.tensor_copy(ptile[:, 0:8], ps[:, 0:8])
                        pts.append(ptile)

                    if "av" not in phases:
                        continue
                    po = pso.tile([65, LC], f32)
                    for j in range(MT // 2):
                        for half in range(2):
                            mt = 2 * j + half
                            nc.tensor.matmul(
                                po[:],
                                lhsT=vt[mt][:, h * 65:(h + 1) * 65],
                                rhs=pts[j][:, half * LC:(half + 1) * LC],
                                start=(mt == 0),
                                stop=(mt == MT - 1),
                            )

                    # normalize: rows 0..63 = O^T, row 64 = denominators
                    rc = rcpp.tile([128, LC], f32)
                    nc.vector.reciprocal(rc[64:65, :], po[64:65, :])
                    # partition_broadcast reads physical partition 0 on HW;
                    # stage the reciprocal row there via a small SBUF DMA
                    rc0 = rcpp.tile([1, LC], f32, name="rc0")
                    nc.sync.dma_start(out=rc0[0:1, :], in_=rc[64:65, :])
                    rb = rcpp.tile([64, LC], f32)
                    nc.gpsimd.partition_broadcast(rb[:], rc0[0:1, :])
                    nt = nrmp.tile([64, LC], f32r)
                    nc.vector.tensor_mul(nt[:], po[0:64, :], rb[:])
                    nc.sync.dma_start(
                        out=ot[h // 2][po_off:po_off + 64, ls], in_=nt[:]
                    )

            # ---- P4: out = O @ woT (direct PSUM -> DRAM DMA) ----
            if "p4" not in phases:
                if "p1" in phases:
                    nc.sync.dma_start(out=out[128:256, 0:512],
                                      in_=qp[0][:, 0:512].bitcast(f32))
                if "av" in phases:
                    nc.sync.dma_start(out=out[0:128, 0:512],
                                      in_=ot[0][:, 0:512].bitcast(f32))
                elif "scores" in phases and pts:
                    nc.sync.dma_start(out=out[0:128, 0:512],
                                      in_=pts[0][:, 0:512].bitcast(f32))
                if "vt" in phases:
                    nc.sync.dma_start(out=out[256:384, 0:260],
                                      in_=vt[0][:].bitcast(f32))
            for lt in (range(MT) if "p4" in phases else []):
                for ec in range(NEC):
                    ps = psmm.tile([128, 512], f32, name="ps4", tag="ps")
                    for t in range(2):
                        nc.tensor.matmul(
                            ps[:],
                            lhsT=ot[t][:, lt * 128:(lt + 1) * 128],
                            rhs=wo_sb[t][:, ec * 512:(ec + 1) * 512],
                            start=(t == 0),
                            stop=(t == 1),
                        )
                    st = stgp.tile([128, 512], f32)
                    nc.vector.tensor_copy(st[:], ps[:])
                    nc.sync.dma_start(
                        out=out[lt * 128:(lt + 1) * 128, ec * 512:(ec + 1) * 512],
                        in_=st[:],
                    )
            _p34.close()

    nc.compile()
    return nc


def _prep_in_maps(x, w_qkv, b_qkv, w_o):
    xT = [np.ascontiguousarray(x[b].T) for b in range(B)]
    in_maps = []
    for core in range(8):
        b, g = divmod(core, 4)
        qs, ks, vs = g * G, D + g * G, 2 * D + g * G
        wqkvT = np.ascontiguousarray(
            np.concatenate(
                [w_qkv[qs:qs + G], w_qkv[ks:ks + G], w_qkv[vs:vs + G]], axis=0
            ).T
        )
        bqk_m = np.ascontiguousarray(
            np.concatenate([b_qkv[qs:qs + G], b_qkv[ks:ks + G]]).reshape(4, 128).T
        )
        woT = np.ascontiguousarray(w_o[:, g * G:(g + 1) * G].T)
        in_maps.append({"xT": xT[b], "wqkvT": wqkvT, "bqk": bqk_m, "woT": woT})
    return in_maps


def kernel(x, w_qkv, b_qkv, w_o, b_o):
    from concourse.bass_utils import run_bass_kernel_spmd

    x = np.asarray(x, dtype=np.float32)
    w_qkv = np.asarray(w_qkv, dtype=np.float32)
    b_qkv = np.asarray(b_qkv, dtype=np.float32)
    w_o = np.asarray(w_o, dtype=np.float32)
    b_o = np.asarray(b_o, dtype=np.float32)

    if "nc" not in _CACHE:
        _CACHE["nc"] = _build()
    nc = _CACHE["nc"]

    in_maps = _prep_in_maps(x, w_qkv, b_qkv, w_o)
    res = run_bass_kernel_spmd(nc, in_maps, list(range(8)))
    partial = np.stack([res.results[i]["out"] for i in range(8)])  # [8, L, D]

    const = w_o @ b_qkv[2 * D:] + b_o  # [D]
    out = partial.reshape(B, 4, L, D).sum(axis=1) + x + const[None, None, :]
    return out.astype(np.float32)
